# revision 6
# baseline (speedup 1.0000x reference)
"""MultiHeadAttention forward on 8 Trainium2 NeuronCores.

Tensor-parallel over heads: each core owns 2 of 16 heads (d_loc=256 of the
2048 QKV output columns, and the matching 256 rows of Wo). Each core
computes a full-shape partial output (bf16); the host sums the 8 partials
and adds bo + bv@Wo.

Problem shape: x [2, 2048, 2048], 16 heads, d_k = 128, fp32.

Schedule (per core), designed to keep the in-order PE stream dense:
  - Scores are computed transposed ST[tk, tq] in [128,1024] PSUM pairs
    (two tk-tiles share one wide PSUM tile) so each ScalarE exp covers
    1024 columns — halving Act's fixed per-instruction overhead.
  - Softmax denominator: Pool engine (nc.gpsimd) folds the 16 exp tiles
    into one [128,512] accumulator; a single ones-matmul per (b,h,ch)
    reduces over partitions (replaces 16 PE matmuls).
  - Attention units software-pipeline: scores(p) | AV(p-1) | filler.
    Fillers keep the PE busy while Act computes exp: batch-1's V
    projection runs inside batch-0's h0 attention units; the output
    projection of chunk c runs inside the attention units of chunk c+1.
  - DMAs are split into 64-128 KB pieces so they spread across the 16
    queues (first matmul at ~3 us instead of ~13 us); y is written bf16.
"""

import functools
from contextlib import ExitStack

import numpy as np

D_MODEL = 2048
NUM_HEADS = 16
DK = 128
B = 2
T = 2048
BT = B * T
N_CORES = 8
H_LOC = NUM_HEADS // N_CORES  # 2 heads per core
D_LOC = H_LOC * DK  # 256
C_TILES = D_MODEL // 128  # 16
TQ = 512  # tq chunk width
NCH = T // TQ  # 4 chunks per batch
TK_TILES = T // 128  # 16
NPAIR = TK_TILES // 2  # 8 score-pairs per attention unit


class OEmitter:
    """Output projection for one (batch, chunk): 16 quanta of 2 matmuls +
    1 DVE copy, one ystage DMA per (t, half). Interleaved into attention
    units as PE filler."""

    def __init__(self, ctx, b, ch, avT, split_dma=1):
        self.ctx = ctx
        self.b = b
        self.avT = avT  # snapshot: O of batch b reads batch b's avT tiles
        self.items = [
            (t, half, q)
            for t in range(ch * 4, (ch + 1) * 4)
            for half in range(2)
            for q in range(2)
        ]
        self.idx = 0
        self.split = split_dma
        self.ys = None

    def emit(self, k):
        c = self.ctx
        nc = c["nc"]
        for _ in range(k):
            if self.idx >= len(self.items):
                return
            t, half, q = self.items[self.idx]
            self.idx += 1
            if q == 0:
                self.ys = c["y_pool"].tile(
                    [128, 1024], c["bf16"], tag="ys", name=f"ys{self.b}_{t}_{half}"
                )
            ps = c["ps_o"].tile([128, TQ], c["f32"], tag="o", name=f"pso{self.b}_{t}_{half}_{q}")
            for d in range(2):
                nc.tensor.matmul(
                    ps,
                    self.avT[d][:, t * 128 : (t + 1) * 128],
                    c["wo_tiles"][d][:, (half * 2 + q) * TQ : (half * 2 + q + 1) * TQ],
                    start=(d == 0),
                    stop=(d == 1),
                )
            nc.vector.tensor_copy(self.ys[:, q * TQ : (q + 1) * TQ], ps)
            if q == 1:
                row0 = self.b * T + t * 128
                w = 1024 // self.split
                for s in range(self.split):
                    nc.sync.dma_start(
                        out=c["y"][
                            row0 : row0 + 128,
                            half * 1024 + s * w : half * 1024 + (s + 1) * w,
                        ],
                        in_=self.ys[:, s * w : (s + 1) * w],
                    )

    def remaining(self):
        return len(self.items) - self.idx


def _body(ctx_stack, tc, xT, wqkv, bqk, wo, y):
    import concourse.bass as bass  # noqa: F401
    from concourse import mybir

    nc = tc.nc
    f32 = mybir.dt.float32
    f32r = mybir.dt.float32r
    bf16 = mybir.dt.bfloat16
    Exp = mybir.ActivationFunctionType.Exp
    inv_sqrt_dk = 1.0 / float(np.sqrt(DK))

    # ---------------- pools ----------------
    wpool = ctx_stack.enter_context(tc.tile_pool(name="wpool", bufs=1))
    x_pool = ctx_stack.enter_context(tc.tile_pool(name="x_pool", bufs=20))
    qkv_pool = ctx_stack.enter_context(tc.tile_pool(name="qkv_pool", bufs=1))
    av_pool = ctx_stack.enter_context(tc.tile_pool(name="av_pool", bufs=1))
    es_pool = ctx_stack.enter_context(tc.tile_pool(name="es_pool", bufs=3))
    acc_pool = ctx_stack.enter_context(tc.tile_pool(name="acc_pool", bufs=2))
    rc_pool = ctx_stack.enter_context(tc.tile_pool(name="rc_pool", bufs=1))
    y_pool = ctx_stack.enter_context(tc.tile_pool(name="y_pool", bufs=2))

    ps_wide = ctx_stack.enter_context(tc.tile_pool(name="ps_wide", bufs=2, space="PSUM"))
    ps_av = ctx_stack.enter_context(tc.tile_pool(name="ps_av", bufs=2, space="PSUM"))
    ps_o = ctx_stack.enter_context(tc.tile_pool(name="ps_o", bufs=2, space="PSUM"))

    # ---------------- resident tensors ----------------
    # Chunk-0 x tiles and weights interleaved, in 64 KB pieces so the 16
    # DMA queues all pull and the first matmul can start at ~3 us.
    w_tiles = []
    xt_pre = []
    for i in range(C_TILES):
        xti = x_pool.tile([128, TQ], f32r, tag="xt", name=f"xtpre{i}")
        for q in range(4):
            nc.sync.dma_start(
                out=xti[:, q * 128 : (q + 1) * 128],
                in_=xT[i * 128 : (i + 1) * 128, q * 128 : (q + 1) * 128],
            )
        xt_pre.append(xti)
        wt = wpool.tile([128, 3 * D_LOC], f32r, tag=f"w{i}", name=f"w{i}")
        for q in range(6):
            nc.sync.dma_start(
                out=wt[:, q * 128 : (q + 1) * 128],
                in_=wqkv[i * 128 : (i + 1) * 128, q * 128 : (q + 1) * 128],
            )
        w_tiles.append(wt)
    bqk_sb = wpool.tile([128, 4], f32, tag="bqk", name="bqk")
    nc.sync.dma_start(out=bqk_sb, in_=bqk[:, :])

    wo_tiles = []
    for d in range(2):
        wot = wpool.tile([128, D_MODEL], f32r, tag=f"wo{d}", name=f"wo{d}")
        for q in range(4):
            nc.sync.dma_start(
                out=wot[:, q * 512 : (q + 1) * 512],
                in_=wo[d * 128 : (d + 1) * 128, q * 512 : (q + 1) * 512],
            )
        wo_tiles.append(wot)

    ones_f = wpool.tile([128, 128], f32, tag="ones_f", name="ones_f")
    nc.vector.memset(ones_f, 1.0)
    ones = wpool.tile([128, 128], f32r, tag="ones", name="ones")
    nc.vector.tensor_copy(ones, ones_f)

    # persistent per-batch tiles, filled in as the schedule runs
    v_cur = {0: [None] * TK_TILES, 1: [None] * TK_TILES}
    qT = kT = avT = None

    ctx = {
        "nc": nc,
        "f32": f32,
        "bf16": bf16,
        "y": y,
        "y_pool": y_pool,
        "ps_o": ps_o,
        "wo_tiles": wo_tiles,
        "avT": None,
    }

    def load_xt_chunk(b, ch):
        t0 = b * T + ch * TQ
        xts = []
        for i in range(C_TILES):
            xti = x_pool.tile([128, TQ], f32r, tag="xt", name=f"xt{b}_{ch}_{i}")
            for q in range(2):
                nc.sync.dma_start(
                    out=xti[:, q * 256 : (q + 1) * 256],
                    in_=xT[i * 128 : (i + 1) * 128, t0 + q * 256 : t0 + (q + 1) * 256],
                )
            xts.append(xti)
        return xts

    def emit_qkt_chunk(b, ch, xt):
        for jp in range(2):
            ps = ps_wide.tile([128, 1024], f32, tag="wide", name=f"psqk{b}_{ch}_{jp}")
            for jj in range(2):
                j = jp * 2 + jj
                psh = ps[:, jj * TQ : (jj + 1) * TQ]
                for i in range(C_TILES):
                    nc.tensor.matmul(
                        psh,
                        w_tiles[i][:, j * 128 : (j + 1) * 128],
                        xt[i],
                        start=(i == 0),
                        stop=(i == C_TILES - 1),
                    )
                dest = (qT[0], qT[1], kT[0], kT[1])[j]
                nc.vector.tensor_scalar_add(
                    dest[:, ch * TQ : (ch + 1) * TQ], psh, bqk_sb[:, j : j + 1]
                )

    def emit_v_group(b, t_idx, xt_chunk):
        ts = t_idx % 4
        ps = ps_o.tile([128, TQ], f32, tag="o", name=f"psv{b}_{t_idx}")
        psv = ps[:, :D_LOC]
        for i in range(C_TILES):
            nc.tensor.matmul(
                psv,
                xt_chunk[i][:, ts * 128 : (ts + 1) * 128],
                w_tiles[i][:, 2 * D_LOC : 3 * D_LOC],
                start=(i == 0),
                stop=(i == C_TILES - 1),
            )
        vt = qkv_pool.tile([128, D_LOC], f32r, tag=f"v{t_idx}", name=f"v{t_idx}_{b}", bufs=2)
        nc.vector.tensor_copy(vt, psv)
        v_cur[b][t_idx] = vt

    def attn_unit(b, h, ch, filler=None, sched=None):
        """One (batch, head, chunk) attention unit, software-pipelined.
        sched[p] quanta of filler work are emitted at score-pair p (9
        entries; last fires after AV(7) before the denominator matmul)."""
        if sched is None:
            sched = [0] * (NPAIR + 1)
        pav = ps_av.tile([128, TQ], f32, tag="av", name=f"pav{b}_{h}_{ch}")
        esacc = acc_pool.tile([128, TQ], f32r, tag="esacc", name=f"esacc{b}_{h}_{ch}")
        es_tiles = [None] * NPAIR

        def av_fold(p):
            es = es_tiles[p]
            for jj in range(2):
                tk = 2 * p + jj
                nc.tensor.matmul(
                    pav,
                    v_cur[b][tk][:, h * 128 : (h + 1) * 128],
                    es[:, jj * TQ : (jj + 1) * TQ],
                    start=(tk == 0),
                    stop=(tk == TK_TILES - 1),
                )
            if p == 0:
                nc.gpsimd.tensor_add(esacc, es[:, 0:TQ], es[:, TQ : 2 * TQ])
            else:
                nc.gpsimd.tensor_add(esacc, esacc, es[:, 0:TQ])
                nc.gpsimd.tensor_add(esacc, esacc, es[:, TQ : 2 * TQ])

        for p in range(NPAIR):
            ps = ps_wide.tile([128, 1024], f32, tag="wide", name=f"pss{b}_{h}_{ch}_{p}")
            for jj in range(2):
                tk = 2 * p + jj
                nc.tensor.matmul(
                    ps[:, jj * TQ : (jj + 1) * TQ],
                    kT[h][:, tk * 128 : (tk + 1) * 128],
                    qT[h][:, ch * TQ : (ch + 1) * TQ],
                    start=True,
                    stop=True,
                )
            es = es_pool.tile([128, 1024], f32r, tag="es", name=f"es{b}_{h}_{ch}_{p}")
            nc.scalar.activation(es, ps, Exp, scale=inv_sqrt_dk)
            es_tiles[p] = es
            if p > 0:
                av_fold(p - 1)
            if filler is not None and sched[p]:
                filler(sched[p])
        av_fold(NPAIR - 1)
        if filler is not None and sched[NPAIR]:
            filler(sched[NPAIR])

        pdn = ps_av.tile([128, TQ], f32, tag="av", name=f"pdn{b}_{h}_{ch}")
        nc.tensor.matmul(pdn, ones[:, 0:128], esacc, start=True, stop=True)
        rc = rc_pool.tile([128, TQ], f32, tag="rc", name=f"rc{b}_{h}_{ch}")
        nc.vector.reciprocal_approx_fast(out=rc, in_=pdn)
        nc.vector.tensor_mul(avT[h][:, ch * TQ : (ch + 1) * TQ], pav, rc)

    # =================== schedule ===================
    o_pending = None  # OEmitter carried across phases
    xt_b1_early = {}

    for b in range(B):
        qT = [
            qkv_pool.tile([128, T], f32r, tag=f"qT{d}", name=f"qT{d}_{b}")
            for d in range(2)
        ]
        kT = [
            qkv_pool.tile([128, T], f32r, tag=f"kT{d}", name=f"kT{d}_{b}")
            for d in range(2)
        ]

        # ---------------- phase P: projections ----------------
        for ch in range(NCH):
            if b == 0 and ch == 0:
                xt = xt_pre
            else:
                xt = load_xt_chunk(b, ch)
            emit_qkt_chunk(b, ch, xt)
            if b == 0 or ch >= 2:  # b1 chunks 0,1 V-projected early (below)
                for ts in range(4):
                    emit_v_group(b, ch * 4 + ts, xt)

        # ---------------- phase A: attention ----------------
        avT = [
            av_pool.tile([128, T], f32r, tag=f"avT{d}", name=f"avT{d}_{b}")
            for d in range(2)
        ]
        ctx["avT"] = avT

        if b == 0:
            # h-outer. Fillers: batch-1 early V projection (chunks 0,1) in
            # the h0 units and (1,0); then O(c) of this batch in unit (1,c+1).
            vq = []  # queue of V-group thunks for b1 chunks 0,1

            def vfill(k, _vq=vq):
                for _ in range(k):
                    if _vq:
                        _vq.pop(0)()

            # 1 V group in (0,0) (late, waits on its xt DMAs), 2 in each of
            # (0,1)..(0,3), 1 in (1,0): 8 total.
            vsched = {
                (0, 0): [0, 0, 0, 0, 0, 0, 1, 0, 0],
                (0, 1): [0, 0, 1, 0, 0, 0, 1, 0, 0],
                (0, 2): [0, 0, 1, 0, 0, 0, 1, 0, 0],
                (0, 3): [0, 0, 1, 0, 0, 0, 1, 0, 0],
                (1, 0): [0, 0, 1, 0, 0, 0, 0, 0, 0],
            }
            units = [(0, c) for c in range(NCH)] + [(1, c) for c in range(NCH)]
            for h, c in units:
                if h == 0 and c in (0, 1):
                    xt_b1_early[c] = load_xt_chunk(1, c)
                    for ts in range(4):
                        t_idx = c * 4 + ts
                        vq.append(
                            functools.partial(emit_v_group, 1, t_idx, xt_b1_early[c])
                        )
                if (h, c) in vsched:
                    attn_unit(b, h, c, vfill, vsched[(h, c)])
                else:
                    oe = OEmitter(ctx, 0, c - 1, avT)
                    attn_unit(b, h, c, oe.emit, [0, 2, 2, 2, 2, 2, 2, 2, 2])
                    assert oe.remaining() == 0
            assert not vq
            o_pending = OEmitter(ctx, 0, NCH - 1, avT)  # O(b0,c3) -> b1 unit (0,0)
        else:
            # ch-outer. O(b0,c3) must drain entirely inside unit (0,0):
            # batch-1's first avT write (end of that unit) would otherwise
            # precede emitted reads of batch-0's avT (WAR on the bufs=1
            # avT tiles). Unit (1,0) has no filler.
            units = [(h, c) for c in range(NCH) for h in range(H_LOC)]
            full = [0, 2, 2, 2, 2, 2, 2, 2, 2]  # 16 quanta
            half = [0, 1, 1, 1, 1, 1, 1, 1, 1]  # 8 quanta
            oe = o_pending
            for h, c in units:
                if c > 0 and h == 0:
                    oe = OEmitter(ctx, 1, c - 1, avT)
                if c == 0:
                    attn_unit(b, h, c, oe.emit, full if h == 0 else None)
                else:
                    attn_unit(b, h, c, oe.emit, half)
                if (h, c) == (0, 0):
                    assert oe.remaining() == 0
            assert oe.remaining() == 0
            # tail: O(b1,c3) with finely split DMAs
            ot = OEmitter(ctx, 1, NCH - 1, avT, split_dma=4)
            ot.emit(16)


@functools.cache
def _build():
    from concourse import bacc
    import concourse.tile as tile
    from concourse import mybir

    nc = bacc.Bacc(
        "TRN2",
        target_bir_lowering=False,
        debug=False,
        enable_asserts=False,
        num_devices=N_CORES,
    )
    f32 = mybir.dt.float32
    f32r = mybir.dt.float32r
    bf16 = mybir.dt.bfloat16
    xT = nc.dram_tensor("xT", [D_MODEL, BT], f32r, kind="ExternalInput").ap()
    wqkv = nc.dram_tensor(
        "wqkv", [D_MODEL, 3 * D_LOC], f32r, kind="ExternalInput"
    ).ap()
    bqk = nc.dram_tensor("bqk", [128, 4], f32, kind="ExternalInput").ap()
    wo = nc.dram_tensor("wo", [D_LOC, D_MODEL], f32r, kind="ExternalInput").ap()
    y = nc.dram_tensor("y", [BT, D_MODEL], bf16, kind="ExternalOutput").ap()

    with tile.TileContext(nc) as tc:
        with ExitStack() as ctx:
            _body(ctx, tc, xT, wqkv, bqk, wo, y)
    nc.compile()
    return nc


def _shard_inputs(x, Wq, bq, Wk, bk, Wv, bv, Wo, bo):
    """Host-side sharding: returns per-core input maps."""
    f = np.float32
    xT = np.ascontiguousarray(np.asarray(x, f).reshape(BT, D_MODEL).T)
    Wq, Wk, Wv, Wo = (np.asarray(a, f) for a in (Wq, Wk, Wv, Wo))
    bq, bk, bv = (np.asarray(a, f) for a in (bq, bk, bv))
    in_maps = []
    for c in range(N_CORES):
        sl = slice(c * D_LOC, (c + 1) * D_LOC)
        wqkv_pad = np.ascontiguousarray(
            np.concatenate([Wq[:, sl], Wk[:, sl], Wv[:, sl]], axis=1)
        )
        bqk_t = np.ascontiguousarray(
            np.stack(
                [
                    bq[sl][:128],
                    bq[sl][128:],
                    bk[sl][:128],
                    bk[sl][128:],
                ],
                axis=1,
            )
        )
        wo_loc = np.ascontiguousarray(Wo[sl, :])
        in_maps.append({"xT": xT, "wqkv": wqkv_pad, "bqk": bqk_t, "wo": wo_loc})
    return in_maps


def _run(in_maps, trace=False, **kwargs):
    from concourse.bass_utils import run_bass_kernel_spmd

    nc = _build()
    return run_bass_kernel_spmd(
        nc, in_maps, core_ids=list(range(N_CORES)), trace=trace, **kwargs
    )


def kernel(x, Wq, bq, Wk, bk, Wv, bv, Wo, bo):
    in_maps = _shard_inputs(x, Wq, bq, Wk, bk, Wv, bv, Wo, bo)
    res = _run(in_maps, trace=False)
    acc = np.zeros((BT, D_MODEL), np.float32)
    for rmap in res.results:
        acc += np.asarray(rmap["y"], dtype=np.float32)
    acc += np.asarray(bo, np.float32)[None, :]
    acc += (np.asarray(bv, np.float32) @ np.asarray(Wo, np.float32))[None, :]
    return acc.reshape(B, T, D_MODEL)


# revision 16
# speedup vs baseline: 1.4225x; 1.4225x over previous
"""MultiHeadAttention forward on 8 Trainium2 NeuronCores.

Tensor-parallel over heads: each core owns 2 of 16 heads (d_loc=256 of the
2048 QKV output columns, and the matching 256 rows of Wo). Each core
computes a full-shape partial output (bf16); the host sums the 8 partials
and adds bo + bv@Wo.

Problem shape: x [2, 2048, 2048], 16 heads, d_k = 128, fp32.

Schedule (per core), designed to keep the in-order PE stream dense:
  - Scores are computed transposed ST[tk, tq] in [128,1024] PSUM pairs
    (two tk-tiles share one wide PSUM tile) so each ScalarE exp covers
    1024 columns — halving Act's fixed per-instruction overhead.
  - Softmax denominator: Pool engine (nc.gpsimd) folds the 16 exp tiles
    into one [128,512] accumulator; a single ones-matmul per (b,h,ch)
    reduces over partitions (replaces 16 PE matmuls).
  - Attention units software-pipeline: scores(p) | AV(p-1) | filler.
    Fillers keep the PE busy while Act computes exp: batch-1's V
    projection runs inside batch-0's h0 attention units; the output
    projection of chunk c runs inside the attention units of chunk c+1.
  - DMAs are split into 64-128 KB pieces so they spread across the 16
    queues (first matmul at ~3 us instead of ~13 us); y is written bf16.
"""

import functools
from contextlib import ExitStack

import numpy as np

D_MODEL = 2048
NUM_HEADS = 16
DK = 128
B = 2
T = 2048
BT = B * T
N_CORES = 8
H_LOC = NUM_HEADS // N_CORES  # 2 heads per core
D_LOC = H_LOC * DK  # 256
C_TILES = D_MODEL // 128  # 16
TQ = 512  # tq chunk width
NCH = T // TQ  # 4 chunks per batch
TK_TILES = T // 128  # 16
NPAIR = TK_TILES // 2  # 8 score-pairs per attention unit


class OEmitter:
    """Output projection for one (batch, chunk): 16 quanta of 2 matmuls +
    1 DVE copy, one ystage DMA per (t, half). Interleaved into attention
    units as PE filler."""

    def __init__(self, ctx, b, ch, avT, split_dma=1):
        self.ctx = ctx
        self.b = b
        self.avT = avT  # snapshot: O of batch b reads batch b's avT tiles
        self.items = [
            (t, half, q)
            for t in range(ch * 4, (ch + 1) * 4)
            for half in range(2)
            for q in range(2)
        ]
        self.idx = 0
        self.split = split_dma
        self.ys = None

    def emit(self, k):
        c = self.ctx
        nc = c["nc"]
        for _ in range(k):
            if self.idx >= len(self.items):
                return
            t, half, q = self.items[self.idx]
            self.idx += 1
            if q == 0:
                self.ys = c["y_pool"].tile(
                    [128, 1024], c["bf16"], tag="ys", name=f"ys{self.b}_{t}_{half}"
                )
            ps = c["ps_o"].tile([128, TQ], c["f32"], tag="o", name=f"pso{self.b}_{t}_{half}_{q}")
            for d in range(2):
                nc.tensor.matmul(
                    ps,
                    self.avT[d][:, t * 128 : (t + 1) * 128],
                    c["wo_tiles"][d][:, (half * 2 + q) * TQ : (half * 2 + q + 1) * TQ],
                    start=(d == 0),
                    stop=(d == 1),
                )
            nc.vector.tensor_copy(self.ys[:, q * TQ : (q + 1) * TQ], ps)
            if q == 1:
                row0 = self.b * T + t * 128
                w = 1024 // self.split
                for s in range(self.split):
                    nc.sync.dma_start(
                        out=c["y"][
                            row0 : row0 + 128,
                            half * 1024 + s * w : half * 1024 + (s + 1) * w,
                        ],
                        in_=self.ys[:, s * w : (s + 1) * w],
                    )

    def remaining(self):
        return len(self.items) - self.idx


def _body(ctx_stack, tc, xT, wqkv, bqk, wo, y):
    import concourse.bass as bass  # noqa: F401
    from concourse import mybir

    nc = tc.nc
    f32 = mybir.dt.float32
    f32r = mybir.dt.float32r
    bf16 = mybir.dt.bfloat16
    Exp = mybir.ActivationFunctionType.Exp
    inv_sqrt_dk = 1.0 / float(np.sqrt(DK))

    # ---------------- pools ----------------
    wpool = ctx_stack.enter_context(tc.tile_pool(name="wpool", bufs=1))
    x_pool = ctx_stack.enter_context(tc.tile_pool(name="x_pool", bufs=48))
    qkv_pool = ctx_stack.enter_context(tc.tile_pool(name="qkv_pool", bufs=1))
    av_pool = ctx_stack.enter_context(tc.tile_pool(name="av_pool", bufs=1))
    es_pool = ctx_stack.enter_context(tc.tile_pool(name="es_pool", bufs=4))
    acc_pool = ctx_stack.enter_context(tc.tile_pool(name="acc_pool", bufs=2))
    rc_pool = ctx_stack.enter_context(tc.tile_pool(name="rc_pool", bufs=1))
    y_pool = ctx_stack.enter_context(tc.tile_pool(name="y_pool", bufs=2))

    ps_wide = ctx_stack.enter_context(tc.tile_pool(name="ps_wide", bufs=2, space="PSUM"))
    ps_av = ctx_stack.enter_context(tc.tile_pool(name="ps_av", bufs=2, space="PSUM"))
    ps_o = ctx_stack.enter_context(tc.tile_pool(name="ps_o", bufs=2, space="PSUM"))

    # ---------------- resident tensors ----------------
    # Chunk-0 x tiles and weights interleaved, in 64 KB pieces so the 16
    # DMA queues all pull and the first matmul can start at ~3 us.
    w_tiles = []
    xt_pre = []
    for i in range(C_TILES):
        xti = x_pool.tile([128, TQ], bf16, tag="xt", name=f"xtpre{i}")
        for q in range(4):
            nc.sync.dma_start(
                out=xti[:, q * 128 : (q + 1) * 128],
                in_=xT[i * 128 : (i + 1) * 128, q * 128 : (q + 1) * 128],
            )
        xt_pre.append(xti)
        wt = wpool.tile([128, 3 * D_LOC], bf16, tag=f"w{i}", name=f"w{i}")
        for q in range(3):
            nc.sync.dma_start(
                out=wt[:, q * 256 : (q + 1) * 256],
                in_=wqkv[i * 128 : (i + 1) * 128, q * 256 : (q + 1) * 256],
            )
        w_tiles.append(wt)
    bqk_sb = wpool.tile([128, 4], f32, tag="bqk", name="bqk")
    nc.sync.dma_start(out=bqk_sb, in_=bqk[:, :])

    wo_tiles = []
    for d in range(2):
        wot = wpool.tile([128, D_MODEL], bf16, tag=f"wo{d}", name=f"wo{d}")
        for q in range(4):
            nc.sync.dma_start(
                out=wot[:, q * 512 : (q + 1) * 512],
                in_=wo[d * 128 : (d + 1) * 128, q * 512 : (q + 1) * 512],
            )
        wo_tiles.append(wot)

    ones_f = wpool.tile([128, 128], f32, tag="ones_f", name="ones_f")
    nc.vector.memset(ones_f, 1.0)
    ones = wpool.tile([128, 128], bf16, tag="ones", name="ones")
    nc.vector.tensor_copy(ones, ones_f)

    # persistent per-batch tiles, filled in as the schedule runs
    v_cur = {0: [None] * TK_TILES, 1: [None] * TK_TILES}
    qT = kT = avT = None

    ctx = {
        "nc": nc,
        "f32": f32,
        "bf16": bf16,
        "y": y,
        "y_pool": y_pool,
        "ps_o": ps_o,
        "wo_tiles": wo_tiles,
        "avT": None,
    }

    def load_xt_chunk(b, ch):
        t0 = b * T + ch * TQ
        xts = []
        for i in range(C_TILES):
            xti = x_pool.tile([128, TQ], bf16, tag="xt", name=f"xt{b}_{ch}_{i}")
            for q in range(2):
                nc.sync.dma_start(
                    out=xti[:, q * 256 : (q + 1) * 256],
                    in_=xT[i * 128 : (i + 1) * 128, t0 + q * 256 : t0 + (q + 1) * 256],
                )
            xts.append(xti)
        return xts

    def emit_qkt_chunk(b, ch, xt):
        for jp in range(2):
            ps = ps_wide.tile([128, 1024], f32, tag="wide", name=f"psqk{b}_{ch}_{jp}")
            for jj in range(2):
                j = jp * 2 + jj
                psh = ps[:, jj * TQ : (jj + 1) * TQ]
                for i in range(C_TILES):
                    nc.tensor.matmul(
                        psh,
                        w_tiles[i][:, j * 128 : (j + 1) * 128],
                        xt[i],
                        start=(i == 0),
                        stop=(i == C_TILES - 1),
                    )
                dest = (qT[0], qT[1], kT[0], kT[1])[j]
                nc.vector.tensor_scalar_add(
                    dest[:, ch * TQ : (ch + 1) * TQ], psh, bqk_sb[:, j : j + 1]
                )

    def emit_v_group(b, t_idx, xt_chunk):
        ts = t_idx % 4
        ps = ps_o.tile([128, TQ], f32, tag="o", name=f"psv{b}_{t_idx}")
        psv = ps[:, :D_LOC]
        for i in range(C_TILES):
            nc.tensor.matmul(
                psv,
                xt_chunk[i][:, ts * 128 : (ts + 1) * 128],
                w_tiles[i][:, 2 * D_LOC : 3 * D_LOC],
                start=(i == 0),
                stop=(i == C_TILES - 1),
            )
        vt = qkv_pool.tile([128, D_LOC], bf16, tag=f"v{t_idx}", name=f"v{t_idx}_{b}", bufs=2)
        nc.vector.tensor_copy(vt, psv)
        v_cur[b][t_idx] = vt

    def attn_unit(b, h, ch, avT_u, filler=None, sched=None, prev_fin=None):
        """One (batch, head, chunk) attention unit, software-pipelined.

        sched[p] quanta of filler work are emitted at score-pair p (9
        entries; last fires after AV(7)). The softmax denominator is
        accumulated off the PE: DVE folds exp halves 0..7, Pool folds
        halves 8..15 (two short parallel chains instead of one long one).
        The unit's own finalize (ones-matmuls + reciprocal + avT write) is
        DEFERRED: this function returns a closure the caller runs inside
        the NEXT unit (at p==2) so the in-order PE never waits on the fold
        chains. prev_fin is the previous unit's closure, run here at p==2.
        """
        if sched is None:
            sched = [0] * (NPAIR + 1)
        pav = ps_av.tile([128, TQ], f32, tag="av", name=f"pav{b}_{h}_{ch}")
        acc_d = acc_pool.tile([128, TQ], bf16, tag="accd", name=f"accd{b}_{h}_{ch}")
        acc_p = acc_pool.tile([128, TQ], bf16, tag="accp", name=f"accp{b}_{h}_{ch}")
        es_tiles = [None] * NPAIR

        def av_fold(p):
            es = es_tiles[p]
            for jj in range(2):
                tk = 2 * p + jj
                nc.tensor.matmul(
                    pav,
                    v_cur[b][tk][:, h * 128 : (h + 1) * 128],
                    es[:, jj * TQ : (jj + 1) * TQ],
                    start=(tk == 0),
                    stop=(tk == TK_TILES - 1),
                )
            # Pool folds pairs 0-2 (it is ~2.5x slower per op; fewer ops,
            # with a head start); DVE folds pairs 3-7 in bf16 2x mode.
            eng, acc = (nc.gpsimd, acc_p) if p < 3 else (nc.vector, acc_d)
            if p in (0, 3):
                eng.tensor_add(acc, es[:, 0:TQ], es[:, TQ : 2 * TQ])
            else:
                eng.tensor_add(acc, acc, es[:, 0:TQ])
                eng.tensor_add(acc, acc, es[:, TQ : 2 * TQ])

        for p in range(NPAIR):
            ps = ps_wide.tile([128, 1024], f32, tag="wide", name=f"pss{b}_{h}_{ch}_{p}")
            for jj in range(2):
                tk = 2 * p + jj
                nc.tensor.matmul(
                    ps[:, jj * TQ : (jj + 1) * TQ],
                    kT[h][:, tk * 128 : (tk + 1) * 128],
                    qT[h][:, ch * TQ : (ch + 1) * TQ],
                    start=True,
                    stop=True,
                )
            es = es_pool.tile([128, 1024], bf16, tag="es", name=f"es{b}_{h}_{ch}_{p}")
            nc.scalar.activation(es, ps, Exp, scale=inv_sqrt_dk)
            es_tiles[p] = es
            if p > 0:
                av_fold(p - 1)
            if p == 2 and prev_fin is not None:
                prev_fin()
            if filler is not None and sched[p]:
                filler(sched[p])
        av_fold(NPAIR - 1)
        if filler is not None and sched[NPAIR]:
            filler(sched[NPAIR])

        def finalize():
            pdn = ps_o.tile([128, TQ], f32, tag="o", name=f"pdn{b}_{h}_{ch}")
            nc.tensor.matmul(pdn, ones[:, 0:128], acc_d, start=True, stop=False)
            nc.tensor.matmul(pdn, ones[:, 0:128], acc_p, start=False, stop=True)
            rc = rc_pool.tile([128, TQ], f32, tag="rc", name=f"rc{b}_{h}_{ch}")
            nc.vector.reciprocal_approx_fast(out=rc, in_=pdn)
            nc.vector.tensor_mul(avT_u[h][:, ch * TQ : (ch + 1) * TQ], pav, rc)

        return finalize

    # =================== schedule ===================
    class WorkQueue:
        def __init__(self):
            self.q = []

        def push(self, thunks):
            self.q.extend(thunks)

        def emit(self, k):
            for _ in range(k):
                if self.q:
                    self.q.pop(0)()

        def __len__(self):
            return len(self.q)

    def o_quanta(oe):
        return [functools.partial(oe.emit, 1) for _ in range(oe.remaining())]

    wq = WorkQueue()
    pend_fin = None  # previous unit's deferred finalize
    avT_b = {}
    xt_b1_early = {}

    for b in range(B):
        qT = [
            qkv_pool.tile([128, T], bf16, tag=f"qT{d}", name=f"qT{d}_{b}")
            for d in range(2)
        ]
        kT = [
            qkv_pool.tile([128, T], bf16, tag=f"kT{d}", name=f"kT{d}_{b}")
            for d in range(2)
        ]

        # ---------------- phase P: projections ----------------
        if b == 1 and pend_fin is not None:
            pend_fin()  # finalize (b0,1,3): its avT feeds O(b0,c3) below
            pend_fin = None
            wq.push(o_quanta(OEmitter(ctx, 0, NCH - 1, avT_b[0])))
        for ch in range(NCH):
            if b == 0 and ch == 0:
                xt = xt_pre
            else:
                xt = load_xt_chunk(b, ch)
            emit_qkt_chunk(b, ch, xt)
            wq.emit(4)
            if b == 0 or ch >= 2:  # b1 chunks 0,1 V-projected early (below)
                for ts in range(4):
                    emit_v_group(b, ch * 4 + ts, xt)
                    wq.emit(2)

        # ---------------- phase A: attention ----------------
        avT = [
            av_pool.tile([128, T], bf16, tag=f"avT{d}", name=f"avT{d}_{b}")
            for d in range(2)
        ]
        avT_b[b] = avT

        if b == 0:
            # h-outer: h0's qT/kT free early. Fillers: batch-1 early V
            # projection (chunks 0,1), then O(b0,c) once finalize(1,c) ran.
            vsched = {
                (0, 0): [0, 0, 0, 0, 0, 0, 1, 0, 0],  # V t0 (late: xt DMAs)
                (0, 1): [0, 0, 1, 0, 0, 0, 1, 0, 0],  # t1,t2
                (0, 2): [0, 0, 1, 0, 0, 0, 1, 0, 0],  # t3,t4
                (0, 3): [0, 0, 1, 0, 0, 0, 1, 0, 0],  # t5,t6
                (1, 0): [0, 0, 1, 0, 0, 0, 0, 0, 0],  # t7
            }
            osched = {
                (1, 1): [0, 0, 0, 2, 2, 2, 1, 1, 0],  # O(b0,c0) from p3
                (1, 2): [0, 1, 1, 1, 1, 1, 1, 1, 1],
                (1, 3): [0, 1, 1, 1, 1, 1, 1, 1, 1],
            }
            units = [(0, c) for c in range(NCH)] + [(1, c) for c in range(NCH)]
            fins = {}
            for h, c in units:
                if h == 0 and c in (0, 1):
                    xt_b1_early[c] = load_xt_chunk(1, c)
                    wq.push(
                        [
                            functools.partial(
                                emit_v_group, 1, c * 4 + ts, xt_b1_early[c]
                            )
                            for ts in range(4)
                        ]
                    )

                def fin_hook(_hc=(h, c)):
                    # runs inside this unit at p==2: emit prev finalize,
                    # then publish O work that finalize unlocked
                    if pend_fin is not None:
                        pend_fin()
                    ph, pc = fins.get("last", (None, None))
                    if ph == 1:  # finalize of (1,pc) unlocks O(b0,pc)
                        wq.push(o_quanta(OEmitter(ctx, 0, pc, avT)))

                sched = vsched.get((h, c)) or osched.get((h, c))
                new_fin = attn_unit(
                    b, h, c, avT, wq.emit, sched, prev_fin=fin_hook
                )
                fins["last"] = (h, c)
                pend_fin = new_fin
            # pend_fin == finalize(b0,1,3); runs at b1 proj start above.
            # O(b0,c3) quanta are published there too (see below).
        else:
            # ch-outer. All O(b0,*) quanta must be emitted before batch-1's
            # first avT write -- i.e. before finalize(b1,0,0), which runs at
            # p==2 of unit (1,0). Unit (0,0) drains the b0 leftovers.
            osched = {
                (0, 0): [0, 1, 1, 1, 1, 1, 1, 1, 1],  # b0 leftovers (8)
                (1, 0): [0, 0, 0, 0, 0, 0, 0, 0, 0],
                (0, 1): [0, 0, 0, 1, 1, 2, 2, 1, 1],  # O(b1,c0) from p3
                (1, 1): [0, 1, 1, 1, 1, 1, 1, 1, 1],
                (0, 2): [0, 0, 0, 1, 1, 2, 2, 1, 1],
                (1, 2): [0, 1, 1, 1, 1, 1, 1, 1, 1],
                (0, 3): [0, 0, 0, 1, 1, 2, 2, 1, 1],
                (1, 3): [0, 1, 1, 1, 1, 1, 1, 1, 1],
            }
            units = [(h, c) for c in range(NCH) for h in range(H_LOC)]
            last = [None]
            for h, c in units:

                def fin_hook(_hc=(h, c)):
                    if pend_fin is not None:
                        pend_fin()
                    if last[0] is not None:
                        ph, pc = last[0]
                        if ph == 1:  # O(b1,pc) unlocked
                            wq.push(o_quanta(OEmitter(ctx, 1, pc, avT)))

                new_fin = attn_unit(
                    b, h, c, avT, wq.emit, osched[(h, c)], prev_fin=fin_hook
                )
                last[0] = (h, c)
                pend_fin = new_fin
                if (h, c) == (0, 0):
                    # all O(b0,*) must be emitted before finalize(b1,0,0)
                    # (WAR on the bufs=1 avT tiles) -- it runs at (1,0) p2
                    assert len(wq) == 0, len(wq)
            assert len(wq) == 0
            # tail: finalize (1,3) now, then O(b1,c3) with finely split DMAs
            pend_fin()
            pend_fin = None
            ot = OEmitter(ctx, 1, NCH - 1, avT, split_dma=4)
            ot.emit(16)


@functools.cache
def _build():
    from concourse import bacc
    import concourse.tile as tile
    from concourse import mybir

    nc = bacc.Bacc(
        "TRN2",
        target_bir_lowering=False,
        debug=False,
        enable_asserts=False,
        num_devices=N_CORES,
    )
    f32 = mybir.dt.float32
    f32r = mybir.dt.float32r
    bf16 = mybir.dt.bfloat16
    xT = nc.dram_tensor("xT", [D_MODEL, BT], bf16, kind="ExternalInput").ap()
    wqkv = nc.dram_tensor(
        "wqkv", [D_MODEL, 3 * D_LOC], bf16, kind="ExternalInput"
    ).ap()
    bqk = nc.dram_tensor("bqk", [128, 4], f32, kind="ExternalInput").ap()
    wo = nc.dram_tensor("wo", [D_LOC, D_MODEL], bf16, kind="ExternalInput").ap()
    y = nc.dram_tensor("y", [BT, D_MODEL], bf16, kind="ExternalOutput").ap()

    with tile.TileContext(nc) as tc:
        with ExitStack() as ctx:
            _body(ctx, tc, xT, wqkv, bqk, wo, y)
    nc.compile()
    return nc


def _shard_inputs(x, Wq, bq, Wk, bk, Wv, bv, Wo, bo):
    """Host-side sharding: returns per-core input maps."""
    import ml_dtypes

    f = np.float32
    b16 = ml_dtypes.bfloat16
    xT = np.ascontiguousarray(np.asarray(x, f).reshape(BT, D_MODEL).T.astype(b16))
    Wq, Wk, Wv, Wo = (np.asarray(a, f) for a in (Wq, Wk, Wv, Wo))
    bq, bk, bv = (np.asarray(a, f) for a in (bq, bk, bv))
    in_maps = []
    for c in range(N_CORES):
        sl = slice(c * D_LOC, (c + 1) * D_LOC)
        wqkv_pad = np.ascontiguousarray(
            np.concatenate([Wq[:, sl], Wk[:, sl], Wv[:, sl]], axis=1).astype(b16)
        )
        bqk_t = np.ascontiguousarray(
            np.stack(
                [
                    bq[sl][:128],
                    bq[sl][128:],
                    bk[sl][:128],
                    bk[sl][128:],
                ],
                axis=1,
            )
        )
        wo_loc = np.ascontiguousarray(Wo[sl, :].astype(b16))
        in_maps.append({"xT": xT, "wqkv": wqkv_pad, "bqk": bqk_t, "wo": wo_loc})
    return in_maps


def _run(in_maps, trace=False, **kwargs):
    from concourse.bass_utils import run_bass_kernel_spmd

    nc = _build()
    return run_bass_kernel_spmd(
        nc, in_maps, core_ids=list(range(N_CORES)), trace=trace, **kwargs
    )


def kernel(x, Wq, bq, Wk, bk, Wv, bv, Wo, bo):
    in_maps = _shard_inputs(x, Wq, bq, Wk, bk, Wv, bv, Wo, bo)
    res = _run(in_maps, trace=False)
    acc = np.zeros((BT, D_MODEL), np.float32)
    for rmap in res.results:
        acc += np.asarray(rmap["y"], dtype=np.float32)
    acc += np.asarray(bo, np.float32)[None, :]
    acc += (np.asarray(bv, np.float32) @ np.asarray(Wo, np.float32))[None, :]
    return acc.reshape(B, T, D_MODEL)
